# revision 9
# baseline (speedup 1.0000x reference)
"""Trainium2 Bass kernel for nn_AutoregressiveDecoder (8-core data parallel).

Strategy (v2 — fp8 DoubleRow):
  - Pure data parallel: B=16384 rows sharded 2048/core across 8 NeuronCores.
  - All matmuls run fp8-e4m3 with MatmulPerfMode.DoubleRow: K=256 packed per
    instruction at 0.5 cycles/row -> 4x fewer PE cycles than bf16.
  - Weights are pre-scaled x64 host-side (they are ~N(0, 0.02^2), below the
    e4m3 normal range); the 1/64 descale folds into the ACT scale operand
    (l1/l2) or the blocked tensor_scalar (l3).
  - Layer-1 over seq_embed is recomputed every step (cheap in fp8-DR); the
    per-step state/one-hot extras join as a small non-DR fp8 matmul whose
    moving operand is a bf16 32x32-stream-transposed state tile (mixed
    bf16-moving x fp8-stationary is supported). A constant-ones row in the
    extras carries the l1 bias, so l1 ACTs are bias-free and merge to
    [128, 1024] reads.
  - Per-row plumbing (bce/clip/select/state scatter) runs in a blocked
    [32 part, 16 blk, 32 slot] bf16 layout (DVE 2x/4x modes).
  - Index preprocessing (perms, one-hot, gathers) and the final scalar loss
    reductions run host-side in numpy (psum of 8x4 partials).
"""

import numpy as np
import ml_dtypes

import concourse.bass as bass
import concourse.bacc as bacc
import concourse.tile as tile
from concourse import mybir
from concourse.bass_utils import run_bass_kernel_spmd

BF16 = mybir.dt.bfloat16
F32 = mybir.dt.float32
FP8 = mybir.dt.float8e4
U8 = mybir.dt.uint8
AF = mybir.ActivationFunctionType
ALU = mybir.AluOpType
DRM = mybir.MatmulPerfMode.DoubleRow
NP_BF16 = ml_dtypes.bfloat16
NP_FP8 = ml_dtypes.float8_e4m3

B, D, H = 16384, 512, 512
NCORES = 8
NB = 512            # macro-tile rows (matmul free dim)
WS = 64.0           # host-side weight scale (descaled by 1/WS on chip)
ALL_PERMS = np.array(
    [[0, 1, 2], [0, 2, 1], [1, 0, 2], [1, 2, 0], [2, 0, 1], [2, 1, 0]], np.int32
)

# blocked-layout slot map (32 slots per 32-row block)
S_P, S_FL, S_ROH, S_ONE, S_F, S_E = 0, 3, 6, 9, 10, 13


def r3(t, s):
    """view a [32, 16*s] tile as [32 p, 16 j, s slots]"""
    return t[:, :].rearrange("p (j s) -> p j s", s=s)


def build_graph(BL):
    """Build the per-core Bass graph. BL = rows per core (multiple of NB)."""
    NM = BL // NB          # macro-tiles per core
    NBLK = NB // 32        # 32-row blocks per macro-tile (16)

    nc = bacc.Bacc("TRN2", target_bir_lowering=False, debug=False,
                   num_devices=NCORES)

    # ---- dram parameters -------------------------------------------------
    seq_d = nc.dram_tensor("seq", [D, BL], FP8, kind="ExternalInput").ap()
    # per-step blocked aux: slots (gtf, gtp, gte, roh0, roh1, roh2) bf16
    gtro_d = nc.dram_tensor("gtro", [96, (BL // 32) * 6], BF16,
                            kind="ExternalInput").ap()
    # slots (mask, roi0, roi1, roi2) uint8
    miro_d = nc.dram_tensor("miro", [96, (BL // 32) * 4], U8,
                            kind="ExternalInput").ap()

    w1p_d = nc.dram_tensor("w1p", [128, 2048], FP8, kind="ExternalInput").ap()
    w1f_d = nc.dram_tensor("w1f", [128, 2048], FP8, kind="ExternalInput").ap()
    w1x_d = nc.dram_tensor("w1x", [48, 512], FP8, kind="ExternalInput").ap()
    w2p_d = nc.dram_tensor("w2p", [128, 1024], FP8, kind="ExternalInput").ap()
    w2f_d = nc.dram_tensor("w2f", [128, 2048], FP8, kind="ExternalInput").ap()
    w3p_d = nc.dram_tensor("w3p", [128, 64], FP8, kind="ExternalInput").ap()
    w3f_d = nc.dram_tensor("w3f", [128, 128], FP8, kind="ExternalInput").ap()
    pb2_d = nc.dram_tensor("pb2v", [128, 2], F32, kind="ExternalInput").ap()
    fb2_d = nc.dram_tensor("fb2v", [128, 4], F32, kind="ExternalInput").ap()
    b3s_d = nc.dram_tensor("b3s", [1, 3], F32, kind="ExternalInput").ap()

    df_d = nc.dram_tensor("df", [BL, 3], BF16, kind="ExternalOutput").ap()
    dp_d = nc.dram_tensor("dp", [BL, 3], BF16, kind="ExternalOutput").ap()
    de_d = nc.dram_tensor("de", [BL, 3], BF16, kind="ExternalOutput").ap()
    # raw per-step MLP outputs, slots (lg s0-2, pf s0-2, pe s0-2)
    lr_d = nc.dram_tensor("lr", [BL, 9], BF16, kind="ExternalOutput").ap()

    v = nc.vector
    sc = nc.scalar
    te = nc.tensor

    def mv(t, j):
        """DR moving view for k-pair j of a [128, >=(j+1)*1024] fp8 tile"""
        return t[:, j * 1024:(j + 1) * 1024].rearrange(
            "p (i n) -> p i n", i=2)

    def drw(t, j, width):
        """DR stationary view for k-pair j: [128, 2, width]"""
        return t[:, j * 2 * width:(j + 1) * 2 * width].rearrange(
            "p (i m) -> p i m", i=2)

    with tile.TileContext(nc) as tc:
        wpool = tc.alloc_tile_pool(name="w", bufs=1)
        pers = tc.alloc_tile_pool(name="pers", bufs=1)
        bigp = tc.alloc_tile_pool(name="big", bufs=3)
        smp = tc.alloc_tile_pool(name="smp", bufs=4)
        psp = tc.alloc_tile_pool(name="psum", bufs=3, space="PSUM")
        pcp = tc.alloc_tile_pool(name="psum3", bufs=2, space="PSUM")

        # ---- load weights (persistent) ----------------------------------
        w1p_sb = wpool.tile([128, 2048], FP8)
        w1f_sb = wpool.tile([128, 2048], FP8)
        w1x_sb = wpool.tile([48, 512], FP8)
        w2p_sb = wpool.tile([128, 1024], FP8)
        w2f_sb = wpool.tile([128, 2048], FP8)
        w3p_sb = wpool.tile([128, 64], FP8)
        w3f_sb = wpool.tile([128, 128], FP8)
        pb2_sb = wpool.tile([128, 2], F32)
        fb2_sb = wpool.tile([128, 4], F32)
        b3s_sb = wpool.tile([1, 3], F32)
        for t, d in ((w1p_sb, w1p_d), (w1f_sb, w1f_d), (w1x_sb, w1x_d),
                     (w2p_sb, w2p_d), (w2f_sb, w2f_d), (w3p_sb, w3p_d),
                     (w3f_sb, w3f_d), (pb2_sb, pb2_d), (fb2_sb, fb2_d),
                     (b3s_sb, b3s_d)):
            nc.scalar.dma_start(t[:, :], d[:, :])
        b3bc = wpool.tile([32, 3], F32)
        nc.gpsimd.partition_broadcast(b3bc[:, :], b3s_sb[:, :])

        # ---- per-macro-tile persistent state ----------------------------
        seqT, sts, lraw = {}, {}, {}
        for mt in range(NM):
            seqT[mt] = pers.tile([128, 2048], FP8, tag=f"seqT{mt}",
                                 name=f"seqT{mt}")
            for k in range(4):
                nc.sync.dma_start(
                    seqT[mt][:, k * NB:(k + 1) * NB],
                    seq_d[k * 128:(k + 1) * 128, mt * NB:(mt + 1) * NB])
            sts[mt] = pers.tile([32, NBLK * 32], BF16, tag=f"st{mt}",
                                name=f"st{mt}")
            v.memset(sts[mt][:, :], 0.0)
            v.memset(r3(sts[mt], 32)[:, :, S_ONE:S_ONE + 1], 1.0)
            lraw[mt] = pers.tile([32, NBLK * 9], BF16, tag=f"lr{mt}",
                                 name=f"lr{mt}")

        # ---- autoregressive steps, macro-interleaved --------------------
        for s in range(3):
            for mt in range(NM):
                st3 = r3(sts[mt], 32)
                gtro_sb = smp.tile([32, NBLK * 6], BF16, tag="gtro")
                nc.sync.dma_start(
                    gtro_sb[:, :],
                    gtro_d[s * 32:(s + 1) * 32,
                           mt * NBLK * 6:(mt + 1) * NBLK * 6])
                g6 = r3(gtro_sb, 6)
                gtf, gtp, gte = g6[:, :, 0:1], g6[:, :, 1:2], g6[:, :, 2:3]
                roh3 = g6[:, :, 3:6]
                miro_sb = smp.tile([32, NBLK * 4], U8, tag="miro")
                nc.sync.dma_start(
                    miro_sb[:, :],
                    miro_d[s * 32:(s + 1) * 32,
                           mt * NBLK * 4:(mt + 1) * NBLK * 4])
                m4 = r3(miro_sb, 4)
                msk, roi3 = m4[:, :, 0:1], m4[:, :, 1:4]

                # bridge to feature-major: merge roh, 32x32 stream transpose
                stx = smp.tile([32, NBLK * 32], BF16, tag="stx")
                v.tensor_copy(stx[:, :], sts[mt][:, :])
                v.tensor_copy(r3(stx, 32)[:, :, S_ROH:S_ROH + 3], roh3)
                exT = smp.tile([64, NB], BF16, tag="exT")
                v.transpose(exT[0:32, :], stx[:, :])
                v.tensor_copy(exT[32:48, :], exT[0:16, :])

                # ---- layer 1: 2 DR matmuls (seq) + extras, per m-chunk --
                h1p = bigp.tile([128, 2048], FP8, tag="h1p")
                h1f = bigp.tile([128, 2048], FP8, tag="h1f")
                for half in range(2):
                    pap = psp.tile([128, 1024], F32, tag="ps")
                    paf = psp.tile([128, 1024], F32, tag="ps")
                    for mh in range(2):
                        m = half * 2 + mh
                        mc = slice(m * 128, (m + 1) * 128)
                        oc = slice(mh * NB, (mh + 1) * NB)
                        for j in range(2):
                            te.matmul(pap[:, oc], drw(w1p_sb, j, 512)[:, :, mc],
                                      mv(seqT[mt], j),
                                      start=(j == 0), stop=False,
                                      perf_mode=DRM)
                        te.matmul(pap[:, oc], w1x_sb[0:10, mc],
                                  exT[0:10, :], start=False, stop=True)
                        for j in range(2):
                            te.matmul(paf[:, oc], drw(w1f_sb, j, 512)[:, :, mc],
                                      mv(seqT[mt], j),
                                      start=(j == 0), stop=False,
                                      perf_mode=DRM)
                        te.matmul(paf[:, oc], w1x_sb[32:48, mc],
                                  exT[32:48, :], start=False, stop=True,
                                  tile_position=(32, 0))
                    hc = slice(half * 1024, (half + 1) * 1024)
                    sc.activation(h1p[:, hc], pap[:, :], AF.Gelu,
                                  scale=1.0 / WS)
                    sc.activation(h1f[:, hc], paf[:, :], AF.Gelu,
                                  scale=1.0 / WS)

                # ---- layer 2 ------------------------------------------
                h2p = bigp.tile([128, 1024], FP8, tag="h2p")
                h2f = bigp.tile([128, 2048], FP8, tag="h2f")
                pa2 = psp.tile([128, 1024], F32, tag="ps")
                for m in range(2):
                    mc = slice(m * 128, (m + 1) * 128)
                    oc = slice(m * NB, (m + 1) * NB)
                    for j in range(2):
                        te.matmul(pa2[:, oc], drw(w2p_sb, j, 256)[:, :, mc],
                                  mv(h1p, j),
                                  start=(j == 0), stop=(j == 1),
                                  perf_mode=DRM)
                for m in range(2):
                    oc = slice(m * NB, (m + 1) * NB)
                    sc.activation(h2p[:, oc], pa2[:, oc], AF.Gelu,
                                  scale=1.0 / WS, bias=pb2_sb[:, m:m + 1])
                for half in range(2):
                    pa3 = psp.tile([128, 1024], F32, tag="ps")
                    for mh in range(2):
                        m = half * 2 + mh
                        mc = slice(m * 128, (m + 1) * 128)
                        oc = slice(mh * NB, (mh + 1) * NB)
                        for j in range(2):
                            te.matmul(pa3[:, oc], drw(w2f_sb, j, 512)[:, :, mc],
                                      mv(h1f, j),
                                      start=(j == 0), stop=(j == 1),
                                      perf_mode=DRM)
                        sc.activation(h2f[:, half * 1024 + oc.start:
                                          half * 1024 + oc.stop],
                                      pa3[:, oc], AF.Gelu, scale=1.0 / WS,
                                      bias=fb2_sb[:, m:m + 1])

                # ---- layer 3 (M padded to 32; col 0 = pres/pf, 1 = pe) --
                pcp_t = pcp.tile([32, NB], F32, tag="pc")
                te.matmul(pcp_t[:, :], drw(w3p_sb, 0, 32),
                          mv(h2p, 0),
                          start=True, stop=True, perf_mode=DRM)
                pcf_t = pcp.tile([32, NB], F32, tag="pc")
                for j in range(2):
                    te.matmul(pcf_t[:, :], drw(w3f_sb, j, 32),
                              mv(h2f, j),
                              start=(j == 0), stop=(j == 1), perf_mode=DRM)

                # bridge back to blocked batch-major
                lgT = smp.tile([32, NB], F32, tag="lgT")
                v.transpose(lgT[:, :], pcp_t[:, :])
                feT = smp.tile([32, NB], F32, tag="feT")
                v.transpose(feT[:, :], pcf_t[:, :])
                lg3 = r3(lgT, 32)
                fe3 = r3(feT, 32)
                lr9 = r3(lraw[mt], 9)
                logit = lr9[:, :, s:s + 1]
                pfr = lr9[:, :, 3 + s:4 + s]
                per = lr9[:, :, 6 + s:7 + s]
                # descale 1/WS + bias -> raw preds (kept for host-side loss)
                v.tensor_scalar(logit, lg3[:, :, 0:1], 1.0 / WS,
                                b3bc[:, 0:1], ALU.mult, ALU.add)
                v.tensor_scalar(pfr, fe3[:, :, 0:1], 1.0 / WS,
                                b3bc[:, 1:2], ALU.mult, ALU.add)
                v.tensor_scalar(per, fe3[:, :, 1:2], 1.0 / WS,
                                b3bc[:, 2:3], ALU.mult, ALU.add)

                # ---- plumbing (all [32, NBLK, 1] APs) -------------------
                pb = smp.tile([32, NBLK * 8], BF16, tag="pb")
                pb3d = r3(pb, 8)
                a_f, a_p, a_e = (pb3d[:, :, 0:1], pb3d[:, :, 1:2],
                                 pb3d[:, :, 2:3])
                sig, pfc, pec = (pb3d[:, :, 3:4], pb3d[:, :, 4:5],
                                 pb3d[:, :, 5:6])

                # sigmoid(l) = 0.5*tanh(0.5*l) + 0.5 (stays on gelu table)
                sc.activation(sig, logit, AF.Tanh, scale=0.5)
                v.tensor_scalar(sig, sig, 0.5, 0.5, ALU.mult, ALU.add)
                v.tensor_scalar(pfc, pfr, -10.0, 10.0, ALU.max, ALU.min)
                v.tensor_scalar(pec, per, -100.0, 100.0, ALU.max, ALU.min)
                v.tensor_copy(a_f, gtf)
                v.copy_predicated(a_f, msk, pfc)
                v.tensor_copy(a_p, gtp)
                v.copy_predicated(a_p, msk, sig)
                v.tensor_copy(a_e, gte)
                v.copy_predicated(a_e, msk, pec)

                # state scatter: slot r = act where roh_r else keep
                for r in range(3):
                    rp = roi3[:, :, r:r + 1]
                    v.copy_predicated(st3[:, :, S_P + r:S_P + r + 1], rp, a_p)
                    v.tensor_max(st3[:, :, S_FL + r:S_FL + r + 1],
                                 st3[:, :, S_FL + r:S_FL + r + 1],
                                 roh3[:, :, r:r + 1])
                    v.copy_predicated(st3[:, :, S_F + r:S_F + r + 1], rp, a_f)
                    v.copy_predicated(st3[:, :, S_E + r:S_E + r + 1], rp, a_e)

                if s == 2:
                    rows = slice(mt * NB, (mt + 1) * NB)
                    nc.gpsimd.dma_start(
                        df_d[rows, :].rearrange("(j p) r -> p j r", p=32),
                        st3[:, :, S_F:S_F + 3])
                    nc.gpsimd.dma_start(
                        dp_d[rows, :].rearrange("(j p) r -> p j r", p=32),
                        st3[:, :, S_P:S_P + 3])
                    nc.gpsimd.dma_start(
                        de_d[rows, :].rearrange("(j p) r -> p j r", p=32),
                        st3[:, :, S_E:S_E + 3])
                    nc.gpsimd.dma_start(
                        lr_d[rows, :].rearrange("(j p) g -> p j g", p=32),
                        lr9[:, :, :])

        for p in (pcp, psp, smp, bigp, pers, wpool):
            p.release()

    nc.compile()
    return nc


# ---------------------------------------------------------------------------
def _pack_dr(w):
    """[K, M] f32 -> DR stationary layout [128, (K//256)*2*M] (pairs, tiles)"""
    K, M = w.shape
    npairs = K // 256
    t = w.reshape(npairs, 2, 128, M).transpose(2, 0, 1, 3)
    return np.ascontiguousarray(t.reshape(128, npairs * 2 * M).astype(NP_FP8))


def prep_inputs(seq_embed, freq, pres, enrich,
                pw1, pb1, pw2, pb2, pw3, pb3,
                fw1, fb1, fw2, fb2, fw3, fb3,
                perm_idx, round_mask, BL):
    """Host-side (numpy) sharding + index preprocessing."""
    f32 = np.float32
    seq = np.asarray(seq_embed, f32)
    perms = ALL_PERMS[np.asarray(perm_idx)]                    # [B,3]
    gtf = np.take_along_axis(np.asarray(freq, f32), perms, 1)   # [B,3] (col=s)
    gtp = np.take_along_axis(np.asarray(pres, f32), perms, 1)
    gte = np.take_along_axis(np.asarray(enrich, f32), perms, 1)
    m = np.take_along_axis(np.asarray(round_mask), perms, 1).astype(f32)
    roh = (perms[:, :, None] == np.arange(3)[None, None, :]).astype(f32)

    pw1, pb1 = np.asarray(pw1, f32), np.asarray(pb1, f32)
    fw1, fb1 = np.asarray(fw1, f32), np.asarray(fb1, f32)
    pw2, pb2 = np.asarray(pw2, f32), np.asarray(pb2, f32)
    fw2, fb2 = np.asarray(fw2, f32), np.asarray(fb2, f32)
    pw3, pb3 = np.asarray(pw3, f32), np.asarray(pb3, f32)
    fw3, fb3 = np.asarray(fw3, f32), np.asarray(fb3, f32)

    # l1 extras stationary [48, 512]: pres rows 0-9, fe rows 32-47.
    # slot order: pres(3), flag(3), roh(3), ones(bias), [fe: freq(3), enr(3)]
    w1x = np.zeros((48, 512), f32)
    w1x[0:9] = pw1[[512, 514, 516, 513, 515, 517, 518, 519, 520]]
    w1x[9] = pb1
    w1x[32:41] = fw1[[513, 517, 521, 515, 519, 523, 524, 525, 526]]
    w1x[41] = fb1
    w1x[42:45] = fw1[[512, 516, 520]]
    w1x[45:48] = fw1[[514, 518, 522]]

    w3p = np.zeros((256, 32), f32)
    w3p[:, 0] = pw3[:, 0]
    w3f = np.zeros((512, 32), f32)
    w3f[:, 0:2] = fw3

    shared = {
        "w1p": _pack_dr(pw1[:512] * WS),
        "w1f": _pack_dr(fw1[:512] * WS),
        "w1x": np.ascontiguousarray((w1x * WS).astype(NP_FP8)),
        "w2p": _pack_dr(pw2 * WS),
        "w2f": _pack_dr(fw2 * WS),
        "w3p": _pack_dr(w3p * WS),
        "w3f": _pack_dr(w3f * WS),
        "pb2v": np.ascontiguousarray(pb2.reshape(2, 128).T),
        "fb2v": np.ascontiguousarray(fb2.reshape(4, 128).T),
        "b3s": np.array([[pb3[0], fb3[0], fb3[1]]], f32),
    }

    in_maps = []
    ncores = seq.shape[0] // BL
    BLKT = BL // 32
    for c in range(ncores):
        rs = slice(c * BL, (c + 1) * BL)
        # blocked layouts: index [s*32+p, j*w + q], b_local = 32*j + p
        ga = np.stack([gtf[rs], gtp[rs], gte[rs],
                       roh[rs, :, 0], roh[rs, :, 1], roh[rs, :, 2]], -1)
        ga = ga.reshape(BLKT, 32, 3, 6).transpose(2, 1, 0, 3)   # [3s,32,J,6]
        mi = np.stack([m[rs], roh[rs, :, 0], roh[rs, :, 1], roh[rs, :, 2]], -1)
        mi = mi.reshape(BLKT, 32, 3, 4).transpose(2, 1, 0, 3)   # [3s,32,J,4]
        in_maps.append(dict(
            seq=np.ascontiguousarray(seq[rs].T.astype(NP_FP8)),
            gtro=np.ascontiguousarray(
                ga.reshape(96, BLKT * 6).astype(NP_BF16)),
            miro=np.ascontiguousarray(
                mi.reshape(96, BLKT * 4).astype(np.uint8)),
            **shared))
    aux = dict(gtf=gtf, gtp=gtp, gte=gte, m=m)
    return in_maps, aux


def assemble(results, aux):
    """Gather per-core outputs; finish the (tiny) loss reductions host-side."""
    f32 = np.float32
    df = np.concatenate([np.asarray(r["df"]) for r in results], 0).astype(f32)
    dp = np.concatenate([np.asarray(r["dp"]) for r in results], 0).astype(f32)
    de = np.concatenate([np.asarray(r["de"]) for r in results], 0).astype(f32)
    lr = np.concatenate([np.asarray(r["lr"]) for r in results], 0).astype(f32)
    lg, pf, pe = lr[:, 0:3], lr[:, 3:6], lr[:, 6:9]
    m, gtf, gtp, gte = aux["m"], aux["gtf"], aux["gtp"], aux["gte"]
    lf = np.sum(np.square(pf - gtf) * m, dtype=np.float64)
    le = np.sum(np.square(pe - gte) * m, dtype=np.float64)
    bce = (np.maximum(lg, 0.0) - lg * gtp
           + np.log1p(np.exp(-np.abs(lg), dtype=np.float64)))
    lp = np.sum(bce * m, dtype=np.float64)
    nm = np.sum(m, dtype=np.float64) + 1e-8
    head = np.array([lf / nm, lp / nm, le / nm], f32)
    return np.concatenate([head, df.ravel(), dp.ravel(), de.ravel()])


_CACHE = {}


def _get_graph(BL):
    if BL not in _CACHE:
        _CACHE[BL] = build_graph(BL)
    return _CACHE[BL]


def _install_profile_hook():
    """Provide antenv.axon_hooks (missing in this image) so trace=True works."""
    import sys, types
    try:
        import antenv.axon_hooks  # noqa: F401
        return
    except ImportError:
        pass
    from trn_agent_boot.trn_boot import _ntff_profile_via_ctypes
    hook = _ntff_profile_via_ctypes('/opt/axon/libaxon_pjrt.so')
    mod = types.ModuleType('antenv.axon_hooks')
    mod._hook = hook
    mod.get_axon_ntff_profile_hook = lambda: mod._hook
    mod.set_axon_ntff_profile_hook = lambda h: setattr(mod, '_hook', h)
    sys.modules['antenv.axon_hooks'] = mod


def run(inputs, trace=False):
    if trace:
        _install_profile_hook()
    BL = inputs["seq_embed"].shape[0] // NCORES
    nc = _get_graph(BL)
    in_maps, aux = prep_inputs(**inputs, BL=BL)
    res = run_bass_kernel_spmd(nc, in_maps, core_ids=list(range(NCORES)),
                               trace=trace)
    out = assemble(res.results, aux)
    return out, res


def kernel(**inputs):
    inputs = {k: np.asarray(v) for k, v in inputs.items()}
    out, _ = run(inputs)
    return out
